# revision 19
# baseline (speedup 1.0000x reference)
# Multi-head attention (B=4, S=2048, D=512, H=8) on 8 Trainium2 NeuronCores.
#
# Sharding: core c handles batch c//2 and query rows [(c%2)*1024, (c%2+1)*1024)
# for all 8 heads over all 2048 keys. Output slices are disjoint -> no
# collectives needed.
#
# Key ideas (layouts chosen so the device never transposes):
#   - host supplies x^T / W^T layouts, bf16 for all matmul operands
#   - masked keys are compacted away on the host: only kept key/value columns
#     (padded to a fixed SKC, multiple of 128) are shipped; padding lanes get
#     the -50 mask bias so exp() underflows to 0. Falls back to dense S keys
#     if a batch keeps more than SKC.
#   - head PAIRS stacked on the 128 partitions; the two K=64 scores matmuls
#     of a pair target PE row-groups 0/64 and run concurrently
#   - scores computed transposed [Sk, Sq]; exp via one [128,1024] ACT call
#     straight from PSUM with the mask folded into the per-partition bias and
#     the 1/sqrt(dk) folded into the scale
#   - p~ @ v via stationary [v_h | 1]: PSUM rows 0..63 accumulate attn^T,
#     row 64 the softmax denominator
#   - normalize: copy PSUM out fast (frees banks), reciprocal, partition-
#     broadcast via DRAM round-trip DMA, multiply
#   - output projection: K=64 per-head contractions, bias via ones-row matmul

import sys
import os

for _p in ("/opt/trn_rl_repo", "/root/.axon_site/_ro/trn_rl_repo"):
    if os.path.isdir(_p) and _p not in sys.path:
        sys.path.append(_p)

import numpy as np

B, S, D, H = 4, 2048, 512, 8
DK = D // H          # 64
N_CORES = 8
SQ = S // 2          # 1024 query rows per core
SKC = 1280           # compacted key capacity (10 tiles of 128)
MASK_BIAS = -50.0

_compiled = {}       # skeys -> Bacc
last_results = None  # BassKernelResults of the most recent run (for test.py)


def _build(skeys):
    import concourse.bass as bass  # noqa: F401
    from concourse import bacc
    import concourse.tile as tile
    import concourse.mybir as mybir

    fp32 = mybir.dt.float32
    bf16 = mybir.dt.bfloat16
    nkt = skeys // 128
    # key-side projection chunks of up to 512 columns (last may be shorter)
    kchunks = []
    off = 0
    while off < skeys:
        w = min(512, skeys - off)
        kchunks.append((off, w))
        off += w

    nc = bacc.Bacc("TRN2", target_bir_lowering=False, debug=False,
                   num_devices=N_CORES)

    xq = nc.dram_tensor("xq", [D, SQ], bf16, kind="ExternalInput")
    xk = nc.dram_tensor("xk", [D, skeys], bf16, kind="ExternalInput")
    xv = nc.dram_tensor("xv", [D, skeys], bf16, kind="ExternalInput")
    wq = nc.dram_tensor("wq", [D, D], bf16, kind="ExternalInput")
    wk = nc.dram_tensor("wk", [D, D], bf16, kind="ExternalInput")
    wv = nc.dram_tensor("wv", [D, D], bf16, kind="ExternalInput")
    wo = nc.dram_tensor("wo", [D, D], bf16, kind="ExternalInput")
    bq = nc.dram_tensor("bq", [128, 4], fp32, kind="ExternalInput")
    bk = nc.dram_tensor("bk", [128, 4], fp32, kind="ExternalInput")
    bv = nc.dram_tensor("bv", [1, D], bf16, kind="ExternalInput")
    bo = nc.dram_tensor("bo", [1, D], bf16, kind="ExternalInput")
    mb = nc.dram_tensor("mb", [128, nkt], fp32, kind="ExternalInput")
    out = nc.dram_tensor("out", [SQ, D], fp32, kind="ExternalOutput")
    rds = nc.dram_tensor("rds", [H, SQ], fp32)  # scratch: 1/denominator

    with tile.TileContext(nc) as tc:
        with (
            tc.tile_pool(name="consts", bufs=1) as consts,
            tc.tile_pool(name="xin", bufs=2) as xin,
            tc.tile_pool(name="qk", bufs=1) as qk,
            tc.tile_pool(name="vp", bufs=1) as vp,
            tc.tile_pool(name="stp", bufs=4) as stp,
            tc.tile_pool(name="small", bufs=2) as small,
            tc.tile_pool(name="osb", bufs=2) as osb,
            tc.tile_pool(name="pst", bufs=2, space="PSUM") as pst,
            tc.tile_pool(name="pout", bufs=2, space="PSUM") as pout,
        ):
            # ---- constant / weight loads ----
            wq_sb = consts.tile([128, 4, D], bf16, tag="wq")
            wk_sb = consts.tile([128, 4, D], bf16, tag="wk")
            wv_sb = consts.tile([128, 4, D], bf16, tag="wv")
            # WoT rows per head at partition base 0 (accumulating matmuls must
            # not switch partition base within one PSUM group)
            wo_sb = consts.tile([DK, H, D], bf16, tag="wo")
            for kc in range(4):
                nc.sync.dma_start(out=wq_sb[:, kc, :],
                                  in_=wq[kc * 128:(kc + 1) * 128, :])
                nc.sync.dma_start(out=wk_sb[:, kc, :],
                                  in_=wk[kc * 128:(kc + 1) * 128, :])
                nc.sync.dma_start(out=wv_sb[:, kc, :],
                                  in_=wv[kc * 128:(kc + 1) * 128, :])
            nc.sync.dma_start(out=wo_sb[:],
                              in_=wo.rearrange("(h p) n -> p h n", p=DK))
            bq_sb = consts.tile([128, 4], fp32, tag="bq")
            bk_sb = consts.tile([128, 4], fp32, tag="bk")
            bv_sb = consts.tile([1, D], bf16, tag="bv")
            bo_sb = consts.tile([1, D], bf16, tag="bo")
            mb_sb = consts.tile([128, nkt], fp32, tag="mb")
            nc.sync.dma_start(out=bq_sb[:], in_=bq[:, :])
            nc.sync.dma_start(out=bk_sb[:], in_=bk[:, :])
            nc.sync.dma_start(out=bv_sb[:], in_=bv[:, :])
            nc.sync.dma_start(out=bo_sb[:], in_=bo[:, :])
            nc.sync.dma_start(out=mb_sb[:], in_=mb[:, :])
            ones_sb = consts.tile([1, 128], bf16, tag="ones")
            nc.vector.memset(ones_sb[:], 1.0)

            qT_sb = qk.tile([128, 4, SQ], bf16, tag="qT")
            kT_sb = qk.tile([128, 4, skeys], bf16, tag="kT")

            def x_chunk(dram, off, w):
                ch = xin.tile([128, 4, 512], bf16, tag="xch")
                nc.sync.dma_start(
                    out=ch[:, :, 0:w],
                    in_=dram[:, off:off + w]
                    .rearrange("(kc p) s -> p kc s", p=128))
                return ch

            # ---- q/k projections (head pairs stacked on partitions) ----
            for qc in range(SQ // 512):
                ch = x_chunk(xq, qc * 512, 512)
                for j in range(4):
                    p = pst.tile([128, SQ], fp32, tag="st")
                    for kc in range(4):
                        nc.tensor.matmul(
                            p[:, 0:512],
                            wq_sb[:, kc, j * 128:(j + 1) * 128],
                            ch[:, kc, :],
                            start=(kc == 0), stop=(kc == 3))
                    nc.scalar.add(qT_sb[:, j, qc * 512:(qc + 1) * 512],
                                  p[:, 0:512], bq_sb[:, j:j + 1])
            for off, w in kchunks:
                ch = x_chunk(xk, off, w)
                for j in range(4):
                    p = pst.tile([128, SQ], fp32, tag="st")
                    for kc in range(4):
                        nc.tensor.matmul(
                            p[:, 0:w],
                            wk_sb[:, kc, j * 128:(j + 1) * 128],
                            ch[:, kc, 0:w],
                            start=(kc == 0), stop=(kc == 3))
                    nc.scalar.add(kT_sb[:, j, off:off + w],
                                  p[:, 0:w], bk_sb[:, j:j + 1])

            # ---- v projection: v = value @ WvT + bv, per head [v_h | 1] ----
            v_sb = vp.tile([128, nkt, H, DK + 1], bf16, tag="v")
            nc.vector.memset(v_sb[:, :, :, DK:DK + 1], 1.0)
            for off, w in kchunks:
                ch = x_chunk(xv, off, w)
                for i in range(w // 128):
                    sk = off // 128 + i
                    p = pst.tile([128, SQ], fp32, tag="st")
                    for kc in range(4):
                        nc.tensor.matmul(
                            p[:, 0:512],
                            ch[:, kc, i * 128:(i + 1) * 128],
                            wv_sb[:, kc, :],
                            start=(kc == 0), stop=False)
                    nc.tensor.matmul(p[:, 0:512], ones_sb[:, 0:128],
                                     bv_sb[:], start=False, stop=True)
                    nc.vector.tensor_copy(
                        out=v_sb[:, sk, :, 0:DK],
                        in_=p[:, 0:512].rearrange("p (h m) -> p h m", h=H))

            # ---- attention, one head pair at a time ----
            # scores for the two heads of a pair use PE row-groups 0/64 and
            # run concurrently; exp is one [128, 1024] ACT call per head
            outTn_sb = qk.tile([DK, H, SQ], bf16, tag="outTn")
            outU_sb = qk.tile([DK + 1, H, SQ], fp32, tag="outU")
            for j in range(4):
                po0 = pout.tile([128, SQ], fp32, tag="po")
                po1 = pout.tile([128, SQ], fp32, tag="po")
                for sk in range(nkt):
                    psA = pst.tile([128, SQ], fp32, tag="st")
                    psB = pst.tile([128, SQ], fp32, tag="st")
                    for qc in range(SQ // 512):
                        nc.tensor.matmul(
                            psA[:, qc * 512:(qc + 1) * 512],
                            kT_sb[0:DK, j, sk * 128:(sk + 1) * 128],
                            qT_sb[0:DK, j, qc * 512:(qc + 1) * 512],
                            start=True, stop=True)
                        nc.tensor.matmul(
                            psB[:, qc * 512:(qc + 1) * 512],
                            kT_sb[DK:128, j, sk * 128:(sk + 1) * 128],
                            qT_sb[DK:128, j, qc * 512:(qc + 1) * 512],
                            start=True, stop=True)
                    stA = stp.tile([128, SQ], bf16, tag="stb")
                    nc.scalar.activation(
                        out=stA[:], in_=psA[:],
                        func=mybir.ActivationFunctionType.Exp,
                        bias=mb_sb[:, sk:sk + 1], scale=0.125)
                    stB = stp.tile([128, SQ], bf16, tag="stb")
                    nc.scalar.activation(
                        out=stB[:], in_=psB[:],
                        func=mybir.ActivationFunctionType.Exp,
                        bias=mb_sb[:, sk:sk + 1], scale=0.125)
                    for qc in range(SQ // 512):
                        nc.tensor.matmul(
                            po0[0:DK + 1, qc * 512:(qc + 1) * 512],
                            v_sb[:, sk, 2 * j, :],
                            stA[:, qc * 512:(qc + 1) * 512],
                            start=(sk == 0), stop=(sk == nkt - 1))
                        nc.tensor.matmul(
                            po1[0:DK + 1, qc * 512:(qc + 1) * 512],
                            v_sb[:, sk, 2 * j + 1, :],
                            stB[:, qc * 512:(qc + 1) * 512],
                            start=(sk == 0), stop=(sk == nkt - 1))
                # evacuate both accumulators quickly (frees PSUM for the
                # next pair); normalization happens after all pairs
                for half, po in ((0, po0), (1, po1)):
                    h = 2 * j + half
                    nc.vector.tensor_copy(out=outU_sb[:, h, :],
                                          in_=po[0:DK + 1, :])
                    nc.sync.dma_start(out=rds[h:h + 1, :],
                                      in_=outU_sb[DK:DK + 1, h, :])

            # ---- batched reciprocal of all H*SQ denominators ----
            # spread over 128 partitions so the 8-cycle/elem divide runs on
            # all lanes at once
            dd = small.tile([128, H * SQ // 128], fp32, tag="dd")
            nc.sync.dma_start(
                out=dd[:],
                in_=rds.rearrange("h q -> (h q)")
                .rearrange("(p f) -> p f", p=128))
            nc.vector.reciprocal(out=dd[:], in_=dd[:])
            nc.sync.dma_start(
                out=rds.rearrange("h q -> (h q)").rearrange("(p f) -> p f",
                                                            p=128),
                in_=dd[:])
            for h in range(H):
                bcn = small.tile([DK, SQ], fp32, tag="bcn")
                nc.gpsimd.dma_start(
                    out=bcn[:],
                    in_=rds[h:h + 1, :].partition_broadcast(DK))
                nc.vector.tensor_mul(out=outTn_sb[:, h, :],
                                     in0=outU_sb[0:DK, h, :], in1=bcn[:])

            # ---- output projection ----
            for sq in range(SQ // 128):
                pf = pout.tile([128, SQ], fp32, tag="po")
                for h in range(H):
                    nc.tensor.matmul(pf[:, 0:512],
                                     outTn_sb[:, h, sq * 128:(sq + 1) * 128],
                                     wo_sb[:, h, :],
                                     start=(h == 0), stop=False)
                nc.tensor.matmul(pf[:, 0:512], ones_sb[:, 0:128], bo_sb[:],
                                 start=False, stop=True)
                ob = osb.tile([128, 512], fp32, tag="ob")
                nc.vector.tensor_copy(out=ob[:], in_=pf[:, 0:512])
                nc.sync.dma_start(out=out[sq * 128:(sq + 1) * 128, :],
                                  in_=ob[:])

    nc.finalize()
    return nc


def _get_nc(skeys):
    if skeys not in _compiled:
        _compiled[skeys] = _build(skeys)
    return _compiled[skeys]


def kernel(query, key, value, key_padding_mask, Wq, bq, Wk, bk, Wv, bv,
           Wo, bo):
    global last_results
    from concourse.bass_utils import run_bass_kernel_spmd
    import ml_dtypes
    bf = ml_dtypes.bfloat16

    query = np.asarray(query, dtype=np.float32)
    key = np.asarray(key, dtype=np.float32)
    value = np.asarray(value, dtype=np.float32)
    mask = np.asarray(key_padding_mask).astype(bool)
    Wq = np.asarray(Wq, dtype=np.float32)
    Wk = np.asarray(Wk, dtype=np.float32)
    Wv = np.asarray(Wv, dtype=np.float32)
    Wo = np.asarray(Wo, dtype=np.float32)
    bqv = np.asarray(bq, dtype=np.float32)
    bkv = np.asarray(bk, dtype=np.float32)
    bvv = np.asarray(bv, dtype=np.float32)
    bov = np.asarray(bo, dtype=np.float32)

    # compact keys: keep only unmasked positions (padded to SKC); dense
    # fallback when a batch keeps more than SKC
    kept = [np.flatnonzero(~mask[b]) for b in range(B)]
    if max(len(k) for k in kept) <= SKC:
        skeys = SKC
        kidx = []
        mbias = []
        for b in range(B):
            idx = np.zeros(SKC, dtype=np.int64)
            idx[:len(kept[b])] = kept[b]
            kidx.append(idx)
            mbias.append(np.where(np.arange(SKC) < len(kept[b]),
                                  np.float32(0.0), np.float32(MASK_BIAS)))
    else:
        skeys = S
        kidx = [None] * B
        mbias = [np.where(mask[b], np.float32(MASK_BIAS), np.float32(0.0))
                 for b in range(B)]

    nc = _get_nc(skeys)
    nkt = skeys // 128

    shared = {
        "wq": np.ascontiguousarray(Wq.T).astype(bf),
        "wk": np.ascontiguousarray(Wk.T).astype(bf),
        "wv": np.ascontiguousarray(Wv.T).astype(bf),
        "wo": np.ascontiguousarray(Wo.T).astype(bf),
        "bq": np.ascontiguousarray(bqv.reshape(4, 128).T),
        "bk": np.ascontiguousarray(bkv.reshape(4, 128).T),
        "bv": bvv.reshape(1, D).astype(bf),
        "bo": bov.reshape(1, D).astype(bf),
    }
    in_maps = []
    for c in range(N_CORES):
        b, qh = divmod(c, 2)
        kc_ = key[b] if kidx[b] is None else key[b][kidx[b]]
        vc_ = value[b] if kidx[b] is None else value[b][kidx[b]]
        qT = np.ascontiguousarray(query[b].T)
        m = {
            "xq": np.ascontiguousarray(
                qT[:, qh * SQ:(qh + 1) * SQ]).astype(bf),
            "xk": np.ascontiguousarray(kc_.T).astype(bf),
            "xv": np.ascontiguousarray(vc_.T).astype(bf),
            "mb": np.ascontiguousarray(mbias[b].reshape(nkt, 128).T),
        }
        m.update(shared)
        in_maps.append(m)

    res = run_bass_kernel_spmd(nc, in_maps, list(range(N_CORES)))
    last_results = res

    out = np.empty((B, S, D), dtype=np.float32)
    for c in range(N_CORES):
        b, qh = divmod(c, 2)
        out[b, qh * SQ:(qh + 1) * SQ, :] = res.results[c]["out"]
    return out


# revision 20
# speedup vs baseline: 1.0540x; 1.0540x over previous
# Multi-head attention (B=4, S=2048, D=512, H=8) on 8 Trainium2 NeuronCores.
#
# Sharding: core c handles batch c//2 and query rows [(c%2)*1024, (c%2+1)*1024)
# for all 8 heads over all 2048 keys. Output slices are disjoint -> no
# collectives needed.
#
# Key ideas (layouts chosen so the device never transposes):
#   - host supplies x^T / W^T layouts, bf16 for all matmul operands
#   - masked keys are compacted away on the host: only kept key/value columns
#     (padded to a fixed SKC, multiple of 128) are shipped; padding lanes get
#     the -50 mask bias so exp() underflows to 0. Falls back to dense S keys
#     if a batch keeps more than SKC.
#   - head PAIRS stacked on the 128 partitions; the two K=64 scores matmuls
#     of a pair target PE row-groups 0/64 and run concurrently
#   - scores computed transposed [Sk, Sq]; exp via one [128,1024] ACT call
#     straight from PSUM with the mask folded into the per-partition bias and
#     the 1/sqrt(dk) folded into the scale
#   - p~ @ v via stationary [v_h | 1]: PSUM rows 0..63 accumulate attn^T,
#     row 64 the softmax denominator
#   - normalize: copy PSUM out fast (frees banks), reciprocal, partition-
#     broadcast via DRAM round-trip DMA, multiply
#   - output projection: K=64 per-head contractions, bias via ones-row matmul

import sys
import os

for _p in ("/opt/trn_rl_repo", "/root/.axon_site/_ro/trn_rl_repo"):
    if os.path.isdir(_p) and _p not in sys.path:
        sys.path.append(_p)

import numpy as np

B, S, D, H = 4, 2048, 512, 8
DK = D // H          # 64
N_CORES = 8
SQ = S // 2          # 1024 query rows per core
SKC = 1280           # compacted key capacity (10 tiles of 128)
MASK_BIAS = -50.0

_compiled = {}       # skeys -> Bacc
last_results = None  # BassKernelResults of the most recent run (for test.py)


def _build(skeys):
    import concourse.bass as bass  # noqa: F401
    from concourse import bacc
    import concourse.tile as tile
    import concourse.mybir as mybir

    fp32 = mybir.dt.float32
    bf16 = mybir.dt.bfloat16
    nkt = skeys // 128
    # key-side projection chunks of up to 512 columns (last may be shorter)
    kchunks = []
    off = 0
    while off < skeys:
        w = min(512, skeys - off)
        kchunks.append((off, w))
        off += w

    nc = bacc.Bacc("TRN2", target_bir_lowering=False, debug=False,
                   num_devices=N_CORES)

    xq = nc.dram_tensor("xq", [D, SQ], bf16, kind="ExternalInput")
    xk = nc.dram_tensor("xk", [D, skeys], bf16, kind="ExternalInput")
    xv = nc.dram_tensor("xv", [D, skeys], bf16, kind="ExternalInput")
    wq = nc.dram_tensor("wq", [D, D], bf16, kind="ExternalInput")
    wk = nc.dram_tensor("wk", [D, D], bf16, kind="ExternalInput")
    wv = nc.dram_tensor("wv", [D, D], bf16, kind="ExternalInput")
    wo = nc.dram_tensor("wo", [D, D], bf16, kind="ExternalInput")
    bq = nc.dram_tensor("bq", [128, 4], fp32, kind="ExternalInput")
    bk = nc.dram_tensor("bk", [128, 4], fp32, kind="ExternalInput")
    bv = nc.dram_tensor("bv", [1, D], bf16, kind="ExternalInput")
    bo = nc.dram_tensor("bo", [1, D], bf16, kind="ExternalInput")
    mb = nc.dram_tensor("mb", [128, nkt], fp32, kind="ExternalInput")
    out = nc.dram_tensor("out", [SQ, D], fp32, kind="ExternalOutput")
    rds = nc.dram_tensor("rds", [H, SQ], fp32)  # scratch: 1/denominator

    with tile.TileContext(nc) as tc:
        with (
            tc.tile_pool(name="consts", bufs=1) as consts,
            tc.tile_pool(name="xin", bufs=2) as xin,
            tc.tile_pool(name="qk", bufs=1) as qk,
            tc.tile_pool(name="vp", bufs=1) as vp,
            tc.tile_pool(name="stp", bufs=4) as stp,
            tc.tile_pool(name="small", bufs=2) as small,
            tc.tile_pool(name="osb", bufs=2) as osb,
            tc.tile_pool(name="pst", bufs=2, space="PSUM") as pst,
            tc.tile_pool(name="pout", bufs=2, space="PSUM") as pout,
        ):
            # ---- constant / weight loads ----
            wq_sb = consts.tile([128, 4, D], bf16, tag="wq")
            wk_sb = consts.tile([128, 4, D], bf16, tag="wk")
            wv_sb = consts.tile([128, 4, D], bf16, tag="wv")
            # WoT rows per head at partition base 0 (accumulating matmuls must
            # not switch partition base within one PSUM group)
            wo_sb = consts.tile([DK, H, D], bf16, tag="wo")
            for kc in range(4):
                nc.sync.dma_start(out=wq_sb[:, kc, :],
                                  in_=wq[kc * 128:(kc + 1) * 128, :])
                nc.sync.dma_start(out=wk_sb[:, kc, :],
                                  in_=wk[kc * 128:(kc + 1) * 128, :])
                nc.sync.dma_start(out=wv_sb[:, kc, :],
                                  in_=wv[kc * 128:(kc + 1) * 128, :])
            nc.sync.dma_start(out=wo_sb[:],
                              in_=wo.rearrange("(h p) n -> p h n", p=DK))
            bq_sb = consts.tile([128, 4], fp32, tag="bq")
            bk_sb = consts.tile([128, 4], fp32, tag="bk")
            bv_sb = consts.tile([1, D], bf16, tag="bv")
            bo_sb = consts.tile([1, D], bf16, tag="bo")
            mb_sb = consts.tile([128, nkt], fp32, tag="mb")
            nc.sync.dma_start(out=bq_sb[:], in_=bq[:, :])
            nc.sync.dma_start(out=bk_sb[:], in_=bk[:, :])
            nc.sync.dma_start(out=bv_sb[:], in_=bv[:, :])
            nc.sync.dma_start(out=bo_sb[:], in_=bo[:, :])
            nc.sync.dma_start(out=mb_sb[:], in_=mb[:, :])
            ones_sb = consts.tile([1, 128], bf16, tag="ones")
            nc.vector.memset(ones_sb[:], 1.0)

            qT_sb = qk.tile([128, 4, SQ], bf16, tag="qT")
            kT_sb = qk.tile([128, 4, skeys], bf16, tag="kT")

            def x_chunk(dram, off, w):
                ch = xin.tile([128, 4, 512], bf16, tag="xch")
                nc.sync.dma_start(
                    out=ch[:, :, 0:w],
                    in_=dram[:, off:off + w]
                    .rearrange("(kc p) s -> p kc s", p=128))
                return ch

            # ---- q/k projections (head pairs stacked on partitions) ----
            for qc in range(SQ // 512):
                ch = x_chunk(xq, qc * 512, 512)
                for j in range(4):
                    p = pst.tile([128, SQ], fp32, tag="st")
                    for kc in range(4):
                        nc.tensor.matmul(
                            p[:, 0:512],
                            wq_sb[:, kc, j * 128:(j + 1) * 128],
                            ch[:, kc, :],
                            start=(kc == 0), stop=(kc == 3))
                    nc.scalar.add(qT_sb[:, j, qc * 512:(qc + 1) * 512],
                                  p[:, 0:512], bq_sb[:, j:j + 1])
            for off, w in kchunks:
                ch = x_chunk(xk, off, w)
                for j in range(4):
                    p = pst.tile([128, SQ], fp32, tag="st")
                    for kc in range(4):
                        nc.tensor.matmul(
                            p[:, 0:w],
                            wk_sb[:, kc, j * 128:(j + 1) * 128],
                            ch[:, kc, 0:w],
                            start=(kc == 0), stop=(kc == 3))
                    nc.scalar.add(kT_sb[:, j, off:off + w],
                                  p[:, 0:w], bk_sb[:, j:j + 1])

            # ---- v projection: v = value @ WvT + bv, per head [v_h | 1] ----
            v_sb = vp.tile([128, nkt, H, DK + 1], bf16, tag="v")
            nc.vector.memset(v_sb[:, :, :, DK:DK + 1], 1.0)
            for off, w in kchunks:
                ch = x_chunk(xv, off, w)
                for i in range(w // 128):
                    sk = off // 128 + i
                    p = pst.tile([128, SQ], fp32, tag="st")
                    for kc in range(4):
                        nc.tensor.matmul(
                            p[:, 0:512],
                            ch[:, kc, i * 128:(i + 1) * 128],
                            wv_sb[:, kc, :],
                            start=(kc == 0), stop=False)
                    nc.tensor.matmul(p[:, 0:512], ones_sb[:, 0:128],
                                     bv_sb[:], start=False, stop=True)
                    nc.vector.tensor_copy(
                        out=v_sb[:, sk, :, 0:DK],
                        in_=p[:, 0:512].rearrange("p (h m) -> p h m", h=H))

            # ---- attention, one head pair at a time ----
            # scores for the two heads of a pair use PE row-groups 0/64 and
            # run concurrently; exp is one [128, 1024] ACT call per head
            outTn_sb = qk.tile([DK, H, SQ], bf16, tag="outTn")
            for j in range(4):
                po0 = pout.tile([128, SQ], fp32, tag="po")
                po1 = pout.tile([128, SQ], fp32, tag="po")
                for sk in range(nkt):
                    psA = pst.tile([128, SQ], fp32, tag="st")
                    psB = pst.tile([128, SQ], fp32, tag="st")
                    for qc in range(SQ // 512):
                        nc.tensor.matmul(
                            psA[:, qc * 512:(qc + 1) * 512],
                            kT_sb[0:DK, j, sk * 128:(sk + 1) * 128],
                            qT_sb[0:DK, j, qc * 512:(qc + 1) * 512],
                            start=True, stop=True)
                        nc.tensor.matmul(
                            psB[:, qc * 512:(qc + 1) * 512],
                            kT_sb[DK:128, j, sk * 128:(sk + 1) * 128],
                            qT_sb[DK:128, j, qc * 512:(qc + 1) * 512],
                            start=True, stop=True)
                    stA = stp.tile([128, SQ], bf16, tag="stb")
                    nc.scalar.activation(
                        out=stA[:], in_=psA[:],
                        func=mybir.ActivationFunctionType.Exp,
                        bias=mb_sb[:, sk:sk + 1], scale=0.125)
                    stB = stp.tile([128, SQ], bf16, tag="stb")
                    nc.scalar.activation(
                        out=stB[:], in_=psB[:],
                        func=mybir.ActivationFunctionType.Exp,
                        bias=mb_sb[:, sk:sk + 1], scale=0.125)
                    for qc in range(SQ // 512):
                        nc.tensor.matmul(
                            po0[0:DK + 1, qc * 512:(qc + 1) * 512],
                            v_sb[:, sk, 2 * j, :],
                            stA[:, qc * 512:(qc + 1) * 512],
                            start=(sk == 0), stop=(sk == nkt - 1))
                        nc.tensor.matmul(
                            po1[0:DK + 1, qc * 512:(qc + 1) * 512],
                            v_sb[:, sk, 2 * j + 1, :],
                            stB[:, qc * 512:(qc + 1) * 512],
                            start=(sk == 0), stop=(sk == nkt - 1))
                # evacuate both accumulators quickly (frees PSUM for the
                # next pair), then normalize off the critical path
                for half, po in ((0, po0), (1, po1)):
                    h = 2 * j + half
                    u65 = small.tile([DK + 1, SQ], fp32, tag="srow")
                    nc.vector.tensor_copy(out=u65[:], in_=po[0:DK + 1, :])
                    nc.vector.reciprocal(out=u65[DK:DK + 1, :],
                                         in_=u65[DK:DK + 1, :])
                    nc.sync.dma_start(out=rds[h:h + 1, :],
                                      in_=u65[DK:DK + 1, :])
                    bcn = small.tile([DK, SQ], fp32, tag="bcn")
                    nc.gpsimd.dma_start(
                        out=bcn[:],
                        in_=rds[h:h + 1, :].partition_broadcast(DK))
                    nc.vector.tensor_mul(out=outTn_sb[:, h, :],
                                         in0=u65[0:DK, :], in1=bcn[:])

            # ---- output projection ----
            for sq in range(SQ // 128):
                pf = pout.tile([128, SQ], fp32, tag="po")
                for h in range(H):
                    nc.tensor.matmul(pf[:, 0:512],
                                     outTn_sb[:, h, sq * 128:(sq + 1) * 128],
                                     wo_sb[:, h, :],
                                     start=(h == 0), stop=False)
                nc.tensor.matmul(pf[:, 0:512], ones_sb[:, 0:128], bo_sb[:],
                                 start=False, stop=True)
                ob = osb.tile([128, 512], fp32, tag="ob")
                nc.vector.tensor_copy(out=ob[:], in_=pf[:, 0:512])
                nc.sync.dma_start(out=out[sq * 128:(sq + 1) * 128, :],
                                  in_=ob[:])

    nc.finalize()
    return nc


def _get_nc(skeys):
    if skeys not in _compiled:
        _compiled[skeys] = _build(skeys)
    return _compiled[skeys]


def kernel(query, key, value, key_padding_mask, Wq, bq, Wk, bk, Wv, bv,
           Wo, bo):
    global last_results
    from concourse.bass_utils import run_bass_kernel_spmd
    import ml_dtypes
    bf = ml_dtypes.bfloat16

    query = np.asarray(query, dtype=np.float32)
    key = np.asarray(key, dtype=np.float32)
    value = np.asarray(value, dtype=np.float32)
    mask = np.asarray(key_padding_mask).astype(bool)
    Wq = np.asarray(Wq, dtype=np.float32)
    Wk = np.asarray(Wk, dtype=np.float32)
    Wv = np.asarray(Wv, dtype=np.float32)
    Wo = np.asarray(Wo, dtype=np.float32)
    bqv = np.asarray(bq, dtype=np.float32)
    bkv = np.asarray(bk, dtype=np.float32)
    bvv = np.asarray(bv, dtype=np.float32)
    bov = np.asarray(bo, dtype=np.float32)

    # compact keys: keep only unmasked positions (padded to SKC); dense
    # fallback when a batch keeps more than SKC
    kept = [np.flatnonzero(~mask[b]) for b in range(B)]
    if max(len(k) for k in kept) <= SKC:
        skeys = SKC
        kidx = []
        mbias = []
        for b in range(B):
            idx = np.zeros(SKC, dtype=np.int64)
            idx[:len(kept[b])] = kept[b]
            kidx.append(idx)
            mbias.append(np.where(np.arange(SKC) < len(kept[b]),
                                  np.float32(0.0), np.float32(MASK_BIAS)))
    else:
        skeys = S
        kidx = [None] * B
        mbias = [np.where(mask[b], np.float32(MASK_BIAS), np.float32(0.0))
                 for b in range(B)]

    nc = _get_nc(skeys)
    nkt = skeys // 128

    shared = {
        "wq": np.ascontiguousarray(Wq.T).astype(bf),
        "wk": np.ascontiguousarray(Wk.T).astype(bf),
        "wv": np.ascontiguousarray(Wv.T).astype(bf),
        "wo": np.ascontiguousarray(Wo.T).astype(bf),
        "bq": np.ascontiguousarray(bqv.reshape(4, 128).T),
        "bk": np.ascontiguousarray(bkv.reshape(4, 128).T),
        "bv": bvv.reshape(1, D).astype(bf),
        "bo": bov.reshape(1, D).astype(bf),
    }
    in_maps = []
    for c in range(N_CORES):
        b, qh = divmod(c, 2)
        kc_ = key[b] if kidx[b] is None else key[b][kidx[b]]
        vc_ = value[b] if kidx[b] is None else value[b][kidx[b]]
        qT = np.ascontiguousarray(query[b].T)
        m = {
            "xq": np.ascontiguousarray(
                qT[:, qh * SQ:(qh + 1) * SQ]).astype(bf),
            "xk": np.ascontiguousarray(kc_.T).astype(bf),
            "xv": np.ascontiguousarray(vc_.T).astype(bf),
            "mb": np.ascontiguousarray(mbias[b].reshape(nkt, 128).T),
        }
        m.update(shared)
        in_maps.append(m)

    res = run_bass_kernel_spmd(nc, in_maps, list(range(N_CORES)))
    last_results = res

    out = np.empty((B, S, D), dtype=np.float32)
    for c in range(N_CORES):
        b, qh = divmod(c, 2)
        out[b, qh * SQ:(qh + 1) * SQ, :] = res.results[c]["out"]
    return out


# revision 21
# speedup vs baseline: 1.0857x; 1.0301x over previous
# Multi-head attention (B=4, S=2048, D=512, H=8) on 8 Trainium2 NeuronCores.
#
# Sharding: core c handles batch c//2 and query rows [(c%2)*1024, (c%2+1)*1024)
# for all 8 heads over all 2048 keys. Output slices are disjoint -> no
# collectives needed.
#
# Key ideas (layouts chosen so the device never transposes):
#   - host supplies x^T / W^T layouts, bf16 for all matmul operands
#   - masked keys are compacted away on the host: only kept key/value columns
#     (padded to a fixed SKC, multiple of 128) are shipped; padding lanes get
#     the -50 mask bias so exp() underflows to 0. Falls back to dense S keys
#     if a batch keeps more than SKC.
#   - head PAIRS stacked on the 128 partitions; the two K=64 scores matmuls
#     of a pair target PE row-groups 0/64 and run concurrently
#   - scores computed transposed [Sk, Sq]; exp via one [128,1024] ACT call
#     straight from PSUM with the mask folded into the per-partition bias and
#     the 1/sqrt(dk) folded into the scale
#   - p~ @ v via stationary [v_h | 1]: PSUM rows 0..63 accumulate attn^T,
#     row 64 the softmax denominator
#   - normalize: copy PSUM out fast (frees banks), reciprocal, partition-
#     broadcast via DRAM round-trip DMA, multiply
#   - output projection: K=64 per-head contractions, bias via ones-row matmul

import sys
import os

for _p in ("/opt/trn_rl_repo", "/root/.axon_site/_ro/trn_rl_repo"):
    if os.path.isdir(_p) and _p not in sys.path:
        sys.path.append(_p)

import numpy as np

B, S, D, H = 4, 2048, 512, 8
DK = D // H          # 64
N_CORES = 8
SQ = S // 2          # 1024 query rows per core
SKC = 1152           # compacted key capacity (9 tiles of 128)
MASK_BIAS = -50.0

_compiled = {}       # skeys -> Bacc
last_results = None  # BassKernelResults of the most recent run (for test.py)


def _build(skeys):
    import concourse.bass as bass  # noqa: F401
    from concourse import bacc
    import concourse.tile as tile
    import concourse.mybir as mybir

    fp32 = mybir.dt.float32
    bf16 = mybir.dt.bfloat16
    nkt = skeys // 128
    # key-side projection chunks of up to 512 columns (last may be shorter)
    kchunks = []
    off = 0
    while off < skeys:
        w = min(512, skeys - off)
        kchunks.append((off, w))
        off += w

    nc = bacc.Bacc("TRN2", target_bir_lowering=False, debug=False,
                   num_devices=N_CORES)

    xq = nc.dram_tensor("xq", [D, SQ], bf16, kind="ExternalInput")
    xk = nc.dram_tensor("xk", [D, skeys], bf16, kind="ExternalInput")
    xv = nc.dram_tensor("xv", [D, skeys], bf16, kind="ExternalInput")
    wq = nc.dram_tensor("wq", [D, D], bf16, kind="ExternalInput")
    wk = nc.dram_tensor("wk", [D, D], bf16, kind="ExternalInput")
    wv = nc.dram_tensor("wv", [D, D], bf16, kind="ExternalInput")
    wo = nc.dram_tensor("wo", [D, D], bf16, kind="ExternalInput")
    bq = nc.dram_tensor("bq", [128, 4], fp32, kind="ExternalInput")
    bk = nc.dram_tensor("bk", [128, 4], fp32, kind="ExternalInput")
    bv = nc.dram_tensor("bv", [1, D], bf16, kind="ExternalInput")
    bo = nc.dram_tensor("bo", [1, D], bf16, kind="ExternalInput")
    mb = nc.dram_tensor("mb", [128, nkt], fp32, kind="ExternalInput")
    out = nc.dram_tensor("out", [SQ, D], fp32, kind="ExternalOutput")
    rds = nc.dram_tensor("rds", [H, SQ], fp32)  # scratch: 1/denominator

    with tile.TileContext(nc) as tc:
        with (
            tc.tile_pool(name="consts", bufs=1) as consts,
            tc.tile_pool(name="xin", bufs=2) as xin,
            tc.tile_pool(name="qk", bufs=1) as qk,
            tc.tile_pool(name="vp", bufs=1) as vp,
            tc.tile_pool(name="stp", bufs=4) as stp,
            tc.tile_pool(name="small", bufs=2) as small,
            tc.tile_pool(name="osb", bufs=2) as osb,
            tc.tile_pool(name="pst", bufs=2, space="PSUM") as pst,
            tc.tile_pool(name="pout", bufs=2, space="PSUM") as pout,
        ):
            # ---- constant / weight loads ----
            wq_sb = consts.tile([128, 4, D], bf16, tag="wq")
            wk_sb = consts.tile([128, 4, D], bf16, tag="wk")
            wv_sb = consts.tile([128, 4, D], bf16, tag="wv")
            # WoT rows per head at partition base 0 (accumulating matmuls must
            # not switch partition base within one PSUM group)
            wo_sb = consts.tile([DK, H, D], bf16, tag="wo")
            for kc in range(4):
                nc.sync.dma_start(out=wq_sb[:, kc, :],
                                  in_=wq[kc * 128:(kc + 1) * 128, :])
                nc.sync.dma_start(out=wk_sb[:, kc, :],
                                  in_=wk[kc * 128:(kc + 1) * 128, :])
                nc.sync.dma_start(out=wv_sb[:, kc, :],
                                  in_=wv[kc * 128:(kc + 1) * 128, :])
            nc.sync.dma_start(out=wo_sb[:],
                              in_=wo.rearrange("(h p) n -> p h n", p=DK))
            bq_sb = consts.tile([128, 4], fp32, tag="bq")
            bk_sb = consts.tile([128, 4], fp32, tag="bk")
            bv_sb = consts.tile([1, D], bf16, tag="bv")
            bo_sb = consts.tile([1, D], bf16, tag="bo")
            mb_sb = consts.tile([128, nkt], fp32, tag="mb")
            nc.sync.dma_start(out=bq_sb[:], in_=bq[:, :])
            nc.sync.dma_start(out=bk_sb[:], in_=bk[:, :])
            nc.sync.dma_start(out=bv_sb[:], in_=bv[:, :])
            nc.sync.dma_start(out=bo_sb[:], in_=bo[:, :])
            nc.sync.dma_start(out=mb_sb[:], in_=mb[:, :])
            ones_sb = consts.tile([1, 128], bf16, tag="ones")
            nc.vector.memset(ones_sb[:], 1.0)

            qT_sb = qk.tile([128, 4, SQ], bf16, tag="qT")
            kT_sb = qk.tile([128, 4, skeys], bf16, tag="kT")

            def x_chunk(dram, off, w):
                ch = xin.tile([128, 4, 512], bf16, tag="xch")
                nc.sync.dma_start(
                    out=ch[:, :, 0:w],
                    in_=dram[:, off:off + w]
                    .rearrange("(kc p) s -> p kc s", p=128))
                return ch

            # ---- q/k projections (head pairs stacked on partitions) ----
            for qc in range(SQ // 512):
                ch = x_chunk(xq, qc * 512, 512)
                for j in range(4):
                    p = pst.tile([128, SQ], fp32, tag="st")
                    for kc in range(4):
                        nc.tensor.matmul(
                            p[:, 0:512],
                            wq_sb[:, kc, j * 128:(j + 1) * 128],
                            ch[:, kc, :],
                            start=(kc == 0), stop=(kc == 3))
                    nc.scalar.add(qT_sb[:, j, qc * 512:(qc + 1) * 512],
                                  p[:, 0:512], bq_sb[:, j:j + 1])
            for off, w in kchunks:
                ch = x_chunk(xk, off, w)
                for j in range(4):
                    p = pst.tile([128, SQ], fp32, tag="st")
                    for kc in range(4):
                        nc.tensor.matmul(
                            p[:, 0:w],
                            wk_sb[:, kc, j * 128:(j + 1) * 128],
                            ch[:, kc, 0:w],
                            start=(kc == 0), stop=(kc == 3))
                    nc.scalar.add(kT_sb[:, j, off:off + w],
                                  p[:, 0:w], bk_sb[:, j:j + 1])

            # ---- v projection: v = value @ WvT + bv, per head [v_h | 1] ----
            v_sb = vp.tile([128, nkt, H, DK + 1], bf16, tag="v")
            nc.vector.memset(v_sb[:, :, :, DK:DK + 1], 1.0)
            for off, w in kchunks:
                ch = x_chunk(xv, off, w)
                for i in range(w // 128):
                    sk = off // 128 + i
                    p = pst.tile([128, SQ], fp32, tag="st")
                    for kc in range(4):
                        nc.tensor.matmul(
                            p[:, 0:512],
                            ch[:, kc, i * 128:(i + 1) * 128],
                            wv_sb[:, kc, :],
                            start=(kc == 0), stop=False)
                    nc.tensor.matmul(p[:, 0:512], ones_sb[:, 0:128],
                                     bv_sb[:], start=False, stop=True)
                    nc.vector.tensor_copy(
                        out=v_sb[:, sk, :, 0:DK],
                        in_=p[:, 0:512].rearrange("p (h m) -> p h m", h=H))

            # ---- attention, one head pair at a time ----
            # scores for the two heads of a pair use PE row-groups 0/64 and
            # run concurrently; exp is one [128, 1024] ACT call per head
            outTn_sb = qk.tile([DK, H, SQ], bf16, tag="outTn")
            for j in range(4):
                po0 = pout.tile([128, SQ], fp32, tag="po")
                po1 = pout.tile([128, SQ], fp32, tag="po")
                for sk in range(nkt):
                    psA = pst.tile([128, SQ], fp32, tag="st")
                    psB = pst.tile([128, SQ], fp32, tag="st")
                    for qc in range(SQ // 512):
                        nc.tensor.matmul(
                            psA[:, qc * 512:(qc + 1) * 512],
                            kT_sb[0:DK, j, sk * 128:(sk + 1) * 128],
                            qT_sb[0:DK, j, qc * 512:(qc + 1) * 512],
                            start=True, stop=True)
                        nc.tensor.matmul(
                            psB[:, qc * 512:(qc + 1) * 512],
                            kT_sb[DK:128, j, sk * 128:(sk + 1) * 128],
                            qT_sb[DK:128, j, qc * 512:(qc + 1) * 512],
                            start=True, stop=True)
                    stA = stp.tile([128, SQ], bf16, tag="stb")
                    nc.scalar.activation(
                        out=stA[:], in_=psA[:],
                        func=mybir.ActivationFunctionType.Exp,
                        bias=mb_sb[:, sk:sk + 1], scale=0.125)
                    stB = stp.tile([128, SQ], bf16, tag="stb")
                    nc.scalar.activation(
                        out=stB[:], in_=psB[:],
                        func=mybir.ActivationFunctionType.Exp,
                        bias=mb_sb[:, sk:sk + 1], scale=0.125)
                    for qc in range(SQ // 512):
                        nc.tensor.matmul(
                            po0[0:DK + 1, qc * 512:(qc + 1) * 512],
                            v_sb[:, sk, 2 * j, :],
                            stA[:, qc * 512:(qc + 1) * 512],
                            start=(sk == 0), stop=(sk == nkt - 1))
                        nc.tensor.matmul(
                            po1[0:DK + 1, qc * 512:(qc + 1) * 512],
                            v_sb[:, sk, 2 * j + 1, :],
                            stB[:, qc * 512:(qc + 1) * 512],
                            start=(sk == 0), stop=(sk == nkt - 1))
                # evacuate both accumulators quickly (frees PSUM for the
                # next pair), then normalize off the critical path
                for half, po in ((0, po0), (1, po1)):
                    h = 2 * j + half
                    u65 = small.tile([DK + 1, SQ], fp32, tag="srow")
                    nc.vector.tensor_copy(out=u65[:], in_=po[0:DK + 1, :])
                    nc.vector.reciprocal(out=u65[DK:DK + 1, :],
                                         in_=u65[DK:DK + 1, :])
                    nc.sync.dma_start(out=rds[h:h + 1, :],
                                      in_=u65[DK:DK + 1, :])
                    bcn = small.tile([DK, SQ], fp32, tag="bcn")
                    nc.gpsimd.dma_start(
                        out=bcn[:],
                        in_=rds[h:h + 1, :].partition_broadcast(DK))
                    nc.vector.tensor_mul(out=outTn_sb[:, h, :],
                                         in0=u65[0:DK, :], in1=bcn[:])

            # ---- output projection ----
            for sq in range(SQ // 128):
                pf = pout.tile([128, SQ], fp32, tag="po")
                for h in range(H):
                    nc.tensor.matmul(pf[:, 0:512],
                                     outTn_sb[:, h, sq * 128:(sq + 1) * 128],
                                     wo_sb[:, h, :],
                                     start=(h == 0), stop=False)
                nc.tensor.matmul(pf[:, 0:512], ones_sb[:, 0:128], bo_sb[:],
                                 start=False, stop=True)
                ob = osb.tile([128, 512], fp32, tag="ob")
                nc.vector.tensor_copy(out=ob[:], in_=pf[:, 0:512])
                nc.sync.dma_start(out=out[sq * 128:(sq + 1) * 128, :],
                                  in_=ob[:])

    nc.finalize()
    return nc


def _get_nc(skeys):
    if skeys not in _compiled:
        _compiled[skeys] = _build(skeys)
    return _compiled[skeys]


def kernel(query, key, value, key_padding_mask, Wq, bq, Wk, bk, Wv, bv,
           Wo, bo):
    global last_results
    from concourse.bass_utils import run_bass_kernel_spmd
    import ml_dtypes
    bf = ml_dtypes.bfloat16

    query = np.asarray(query, dtype=np.float32)
    key = np.asarray(key, dtype=np.float32)
    value = np.asarray(value, dtype=np.float32)
    mask = np.asarray(key_padding_mask).astype(bool)
    Wq = np.asarray(Wq, dtype=np.float32)
    Wk = np.asarray(Wk, dtype=np.float32)
    Wv = np.asarray(Wv, dtype=np.float32)
    Wo = np.asarray(Wo, dtype=np.float32)
    bqv = np.asarray(bq, dtype=np.float32)
    bkv = np.asarray(bk, dtype=np.float32)
    bvv = np.asarray(bv, dtype=np.float32)
    bov = np.asarray(bo, dtype=np.float32)

    # compact keys: keep only unmasked positions (padded to SKC); dense
    # fallback when a batch keeps more than SKC
    kept = [np.flatnonzero(~mask[b]) for b in range(B)]
    if max(len(k) for k in kept) <= SKC:
        skeys = SKC
        kidx = []
        mbias = []
        for b in range(B):
            idx = np.zeros(SKC, dtype=np.int64)
            idx[:len(kept[b])] = kept[b]
            kidx.append(idx)
            mbias.append(np.where(np.arange(SKC) < len(kept[b]),
                                  np.float32(0.0), np.float32(MASK_BIAS)))
    else:
        skeys = S
        kidx = [None] * B
        mbias = [np.where(mask[b], np.float32(MASK_BIAS), np.float32(0.0))
                 for b in range(B)]

    nc = _get_nc(skeys)
    nkt = skeys // 128

    shared = {
        "wq": np.ascontiguousarray(Wq.T).astype(bf),
        "wk": np.ascontiguousarray(Wk.T).astype(bf),
        "wv": np.ascontiguousarray(Wv.T).astype(bf),
        "wo": np.ascontiguousarray(Wo.T).astype(bf),
        "bq": np.ascontiguousarray(bqv.reshape(4, 128).T),
        "bk": np.ascontiguousarray(bkv.reshape(4, 128).T),
        "bv": bvv.reshape(1, D).astype(bf),
        "bo": bov.reshape(1, D).astype(bf),
    }
    in_maps = []
    for c in range(N_CORES):
        b, qh = divmod(c, 2)
        kc_ = key[b] if kidx[b] is None else key[b][kidx[b]]
        vc_ = value[b] if kidx[b] is None else value[b][kidx[b]]
        qT = np.ascontiguousarray(query[b].T)
        m = {
            "xq": np.ascontiguousarray(
                qT[:, qh * SQ:(qh + 1) * SQ]).astype(bf),
            "xk": np.ascontiguousarray(kc_.T).astype(bf),
            "xv": np.ascontiguousarray(vc_.T).astype(bf),
            "mb": np.ascontiguousarray(mbias[b].reshape(nkt, 128).T),
        }
        m.update(shared)
        in_maps.append(m)

    res = run_bass_kernel_spmd(nc, in_maps, list(range(N_CORES)))
    last_results = res

    out = np.empty((B, S, D), dtype=np.float32)
    for c in range(N_CORES):
        b, qh = divmod(c, 2)
        out[b, qh * SQ:(qh + 1) * SQ, :] = res.results[c]["out"]
    return out
